# revision 24
# baseline (speedup 1.0000x reference)
"""Trainium2 Bass kernel for the contextual channel-attention transformer block.

Contract: kernel(**inputs) takes the FULL unsharded inputs
(x: (8,512,64,64) f32, Wq/Wk/Wv: (512,512) f32, gamma: (1,) f32) and
returns the FULL (8,512,64,64) f32 output.  Internally the batch is
data-parallel across 8 NeuronCores (one batch element per core).

Per-core algorithm (fp8-e4m3 DoubleRow matmuls, fp32 PSUM accumulation):
  Gx   = X @ X.T     upper-triangular blocks fp8 DR + 6 PE transposes
  M3q  = Gx Wq^T, M3k = Gx Wk^T   (fp8 DR)   psum = M3/2
  |Q|^2, |K|^2 = diag(W M3) via ones-matmul colsums
  m3q' = (M3q/128) * rq[c]  (rq = 1/(sqrt8 |Q|) row-broadcast)
  G^T  = (16Wk) m3q' (fp8 DR) = G^T rq/8 ; softmax w/ rk folded into
  ACT scale/bias ; msm8 = 64*softmax ; A^T = (16Wv)^T msm8 (fp8 DR)
  out  = at8 @ X8 (fp8 DR) = 64*out ; y = x_bf16 + f*out in one DVE op
"""

import os
import sys

for _p in ("/opt/trn_rl_repo", "/root/.axon_site/_ro/trn_rl_repo"):
    if os.path.isdir(_p) and _p not in sys.path:
        sys.path.insert(0, _p)

import ml_dtypes
import numpy as np

import concourse.bass as bass
import concourse.tile as tile
from concourse import bacc, bass_utils, mybir

B, C, HH, WW = 8, 512, 64, 64
N = HH * WW          # 4096 spatial positions
G = C // 128         # 4 channel groups of 128
NP = N // 256        # 16 spatial pair-chunks (2x128 for DoubleRow)
NJ = N // 512        # 8 output chunks (512 spatial each)
EPS = 1e-6
FP32 = mybir.dt.float32
BF16 = mybir.dt.bfloat16
F8 = mybir.dt.float8e4
F8MAX = 240.0        # TRN FP8_EXP4 max normal
DR = mybir.MatmulPerfMode.DoubleRow
XT_CHUNKS = [(0, 2), (2, 2), (4, 4), (8, 8)]  # (pair_start, npairs)

_CACHE = {}


def _build_nc():
    nc = bacc.Bacc("TRN2", target_bir_lowering=False)

    xt_d = nc.dram_tensor("xt", [N, C], F8, kind="ExternalInput")     # X^T fp8
    xh8_d = nc.dram_tensor("xh8", [C, N], F8, kind="ExternalInput")   # X fp8
    xhb_d = nc.dram_tensor("xhb", [C, N], BF16, kind="ExternalInput")
    wq_d = nc.dram_tensor("wq8", [C, C], F8, kind="ExternalInput")    # 16Wq^T
    wk_d = nc.dram_tensor("wk8", [C, C], F8, kind="ExternalInput")    # 16Wk^T
    wv_d = nc.dram_tensor("wv8", [C, C], F8, kind="ExternalInput")    # 16Wv
    gcol_d = nc.dram_tensor("gamma_col", [128, 1], FP32, kind="ExternalInput")
    id_d = nc.dram_tensor("ident8", [128, 128], F8, kind="ExternalInput")
    y_d = nc.dram_tensor("y", [C, N], BF16, kind="ExternalOutput")

    xt_v = xt_d.ap().rearrange("(i p) c -> p i c", p=128)     # [128, 32, C]
    xh8_v = xh8_d.ap().rearrange("(g p) n -> p g n", p=128)   # [128, G, N]
    xhb_v = xhb_d.ap().rearrange("(g p) n -> p g n", p=128)
    wq_v = wq_d.ap().rearrange("(g p) o -> p g o", p=128)     # [128, G, C]
    wk_v = wk_d.ap().rearrange("(g p) o -> p g o", p=128)
    wv_v = wv_d.ap().rearrange("(g p) o -> p g o", p=128)
    y_v = y_d.ap().rearrange("(g p) n -> p g n", p=128)

    MUL = mybir.AluOpType.mult
    ADD = mybir.AluOpType.add
    MIN = mybir.AluOpType.min
    AX = mybir.AxisListType.X
    Exp = mybir.ActivationFunctionType.Exp
    Ln = mybir.ActivationFunctionType.Ln
    Copy = mybir.ActivationFunctionType.Copy

    with tile.TileContext(nc) as tc:
        with (
            tc.tile_pool(name="consts", bufs=1) as cpool,
            tc.tile_pool(name="weights", bufs=1) as wpool,
            tc.tile_pool(name="xt", bufs=1) as xtpool,
            tc.tile_pool(name="xbig", bufs=1) as xbigpool,
            tc.tile_pool(name="gram", bufs=1) as gpool,
            tc.tile_pool(name="small", bufs=2) as spool,
            tc.tile_pool(name="mid", bufs=3) as mpool,
            tc.tile_pool(name="msm", bufs=1) as msmpool,
            tc.tile_pool(name="outs", bufs=4) as opool,
        ):
            # ---- input DMAs on SP queue (xt first: Gram is the head) -----
            xtp = []
            for ci, (p0, np_) in enumerate(XT_CHUNKS):
                t = xtpool.tile([128, 2 * np_, C], F8, tag=f"xt{ci}",
                                name=f"xt{ci}")
                nc.sync.dma_start(t[:], xt_v[:, 2 * p0:2 * (p0 + np_), :])
                xtp.append(t)

            def xt_pair(i):
                """AP [128, 2, C] for spatial pair i."""
                for (p0, np_), t in zip(XT_CHUNKS, xtp):
                    if p0 <= i < p0 + np_:
                        return t[:, 2 * (i - p0):2 * (i - p0) + 2, :]
                raise AssertionError(i)

            wq = wpool.tile([128, G, C], F8, tag="wq")
            wk = wpool.tile([128, G, C], F8, tag="wk")
            wv = wpool.tile([128, G, C], F8, tag="wv")
            nc.sync.dma_start(wq[:], wq_v)
            nc.sync.dma_start(wk[:], wk_v)
            nc.sync.dma_start(wv[:], wv_v)

            xh8 = xbigpool.tile([128, G, N], F8, tag="xh8")
            nc.sync.dma_start(xh8[:], xh8_v)
            ident = cpool.tile([128, 128], F8, tag="ident")
            nc.sync.dma_start(ident[:], id_d.ap())
            gamma_col = cpool.tile([128, 1], FP32, tag="gamma_col")
            nc.sync.dma_start(gamma_col[:], gcol_d.ap())
            xhb = xbigpool.tile([128, G, N], BF16, tag="xhb")
            nc.sync.dma_start(xhb[:], xhb_v)

            # ---- on-device constants + PE warmup -------------------------
            ones_col = cpool.tile([128, 1], BF16, tag="ones_col")
            nc.vector.memset(ones_col[:], 1.0)
            ones8 = cpool.tile([128, 1], F8, tag="ones8")
            nc.vector.memset(ones8[:], 1.0)
            ones_row = cpool.tile([1, C], BF16, tag="ones_row")
            nc.vector.memset(ones_row[:], 1.0)
            wrm = cpool.tile([128, C], BF16, tag="wrm")
            nc.vector.memset(wrm[:], 0.001)
            # per-partition Ln input scale: row 0 (sqq) 1.0, row 32 (sqk) 1/4096
            scale_col = cpool.tile([33, 1], FP32, tag="scale_col")
            nc.vector.memset(scale_col[:], 1.0)
            nc.vector.memset(scale_col[32:33, :], 1.0 / 4096.0)

            # ---- Gram: Gx = X X^T upper blocks, fp8 DR, + transposes -----
            gx8 = gpool.tile([128, G, C], F8, tag="gx8")
            with tc.tile_pool(name="psG", bufs=1, space="PSUM") as psG:
                scr = psG.tile([128, 512], FP32, tag="scr", name="warm")
                for i in range(12):
                    nc.tensor.matmul(scr[:], wrm[:, 0:128], wrm[:],
                                     start=(i == 0), stop=(i == 11))
                for cg in range(G):
                    gx_ps = psG.tile([128, C], FP32, tag="gx", bufs=2,
                                     name=f"gx{cg}")
                    for i in range(NP):
                        nc.tensor.matmul(
                            gx_ps[:, cg * 128:],
                            xt_pair(i)[:, :, cg * 128:(cg + 1) * 128],
                            xt_pair(i)[:, :, cg * 128:],
                            start=(i == 0), stop=(i == NP - 1),
                            perf_mode=DR)
                    nc.scalar.activation(gx8[:, cg, cg * 128:],
                                         gx_ps[:, cg * 128:], Copy,
                                         scale=1.0 / 32.0)
                # lower blocks by PE transpose of the upper ones
                for dg in range(1, G):
                    for cg in range(dg):
                        tp_ps = psG.tile([128, 256], F8, tag="tp", bufs=3,
                                         name=f"tp{dg}_{cg}")
                        nc.tensor.transpose(
                            tp_ps[:, 0:256:2],
                            gx8[:, cg, dg * 128:(dg + 1) * 128],
                            ident[:])
                        nc.scalar.activation(gx8[:, dg, cg * 128:(cg + 1) * 128],
                                             tp_ps[:, 0:256:2], Copy)

            m3q16 = gpool.tile([128, G, C], BF16, tag="m3q16")
            m3q8 = gpool.tile([128, G, C], F8, tag="m3q8")
            tqs, tks = [], []

            with tc.tile_pool(name="psN", bufs=1, space="PSUM") as psN:
                # packed norms: row 0 = sqq = 8|Q|^2, row 32 = sqk = 8|K|^2
                nsum = psN.tile([33, C], FP32, tag="nsum", name="nsum")
                sqq = nsum[0:1, :]
                sqk = nsum[32:33, :]

                # ---- M3q = Gx Wq^T (psum = M3q/2) ------------------------
                with tc.tile_pool(name="psQ", bufs=1, space="PSUM") as psQ:
                    for cg in range(G):
                        q_ps = psQ.tile([128, C], FP32, tag="q", bufs=G,
                                        name=f"q{cg}")
                        for t in range(2):
                            nc.tensor.matmul(
                                q_ps[:],
                                gx8[:, 2 * t:2 * t + 2,
                                    cg * 128:(cg + 1) * 128],
                                wq[:, 2 * t:2 * t + 2, :],
                                start=(t == 0), stop=(t == 1), perf_mode=DR)
                        tq = mpool.tile([128, C], BF16, tag="tq", bufs=G,
                                        name=f"tq{cg}")
                        nc.vector.tensor_tensor(tq[:], wq[:, cg, :], q_ps[:],
                                                op=MUL)
                        tqs.append(tq)
                        nc.scalar.activation(m3q16[:, cg, :], q_ps[:], Copy,
                                             scale=1.0 / 64.0)
                    for cg in range(G):
                        nc.tensor.matmul(sqq, ones_col[:], tqs[cg][:],
                                         start=(cg == 0), stop=(cg == G - 1))

                # ---- M3k = Gx Wk^T; tk for |K|^2 -------------------------
                with tc.tile_pool(name="psK", bufs=1, space="PSUM") as psK:
                    for cg in range(G):
                        k_ps = psK.tile([128, C], FP32, tag="k", bufs=G,
                                        name=f"k{cg}")
                        for t in range(2):
                            nc.tensor.matmul(
                                k_ps[:],
                                gx8[:, 2 * t:2 * t + 2,
                                    cg * 128:(cg + 1) * 128],
                                wk[:, 2 * t:2 * t + 2, :],
                                start=(t == 0), stop=(t == 1), perf_mode=DR)
                        tk = mpool.tile([128, C], BF16, tag="tk", bufs=G,
                                        name=f"tk{cg}")
                        nc.vector.tensor_tensor(tk[:], wk[:, cg, :], k_ps[:],
                                                op=MUL)
                        tks.append(tk)
                    for cg in range(G):
                        nc.tensor.matmul(sqk, ones_col[:], tks[cg][:],
                                         start=(cg == 0), stop=(cg == G - 1))

                # single Ln + single Exp over the packed [33, C] tile
                lnp = spool.tile([33, C], FP32, tag="lnp")
                nc.scalar.activation(lnp[:], nsum[:], Ln, scale=scale_col[:])
                ep = spool.tile([33, C], BF16, tag="ep")
                nc.scalar.activation(ep[:], lnp[:], Exp, scale=-0.5)
                rq_bf = ep[0:1, :]       # 1/(sqrt8 |Q|)
                zrow = ep[32:33, :]      # 64*rk = 22.6/|K|

                zs = []
                with tc.tile_pool(name="psS", bufs=1, space="PSUM") as psS:
                    bq_ps = psS.tile([128, C], FP32, tag="bq_ps", name="bq_ps")
                    nc.tensor.matmul(bq_ps[:], ones_row[:, 0:128], rq_bf,
                                     start=True, stop=True)
                    # z columns (64*rk per d) via tiny PE transposes
                    for dg in range(G):
                        z_ps = psS.tile([128, 1], BF16, tag="ztp", bufs=2,
                                        name=f"ztp{dg}")
                        nc.tensor.transpose(
                            z_ps[:], zrow[:, dg * 128:(dg + 1) * 128],
                            ones_col[32:33, 0:1])
                        z = spool.tile([128, 1], FP32, tag="z", bufs=G,
                                       name=f"z{dg}")
                        nc.vector.tensor_copy(z[:], z_ps[:])
                        zs.append(z)

                    # m3q8 = (M3q/128) * rq[c]  (fp8, col-scaled; bq from PSUM)
                    for cg in range(G):
                        nc.vector.tensor_tensor(m3q8[:, cg, :],
                                                m3q16[:, cg, :],
                                                bq_ps[:], op=MUL)

            # ---- per-dg: G^T -> softmax -> msm8; A^T over dg pairs -------
            msm = msmpool.tile([128, G, C], F8, tag="msm")
            at8 = msmpool.tile([128, G, C], F8, tag="at8")
            fcols = []
            with tc.tile_pool(name="psB", bufs=1, space="PSUM") as psB:
                at_ps = [psB.tile([128, C], FP32, tag="at", bufs=G,
                                  name=f"at{eg}") for eg in range(G)]
                for dg in range(G):
                    g_ps = psB.tile([128, C], FP32, tag="g_ps", bufs=G,
                                    name=f"g_ps{dg}")
                    for t in range(2):
                        nc.tensor.matmul(
                            g_ps[:],
                            wk[:, 2 * t:2 * t + 2, dg * 128:(dg + 1) * 128],
                            m3q8[:, 2 * t:2 * t + 2, :],
                            start=(t == 0), stop=(t == 1), perf_mode=DR)
                    mn0 = spool.tile([128, 1], FP32, tag="mn0")
                    nc.vector.tensor_reduce(mn0[:], g_ps[:], axis=AX, op=MIN)
                    mn = spool.tile([128, 1], FP32, tag="mn")
                    nc.vector.tensor_tensor(mn[:], mn0[:], zs[dg][:], op=MUL)
                    # den4 = (1+eps-mn)/4 ; r4 = 4/(1+eps-mn) = r*INV_H
                    den4 = spool.tile([128, 1], FP32, tag="den4")
                    nc.vector.tensor_scalar(den4[:], mn[:], -0.25,
                                            0.25 * (1.0 + EPS),
                                            op0=MUL, op1=ADD)
                    r4 = spool.tile([128, 1], FP32, tag="r4")
                    nc.vector.reciprocal(r4[:], den4[:])
                    sv = spool.tile([128, 1], FP32, tag="sv")
                    nc.vector.tensor_tensor(sv[:], r4[:], zs[dg][:], op=MUL)
                    bv = spool.tile([128, 1], FP32, tag="bv")
                    nc.vector.tensor_scalar(bv[:], r4[:], -1.0, 1.0,
                                            op0=MUL, op1=ADD)
                    e = mpool.tile([128, C], BF16, tag="e")
                    se = spool.tile([128, 1], FP32, tag="se")
                    nc.scalar.activation(e[:], g_ps[:], Exp,
                                         bias=bv[:], scale=sv[:],
                                         accum_out=se[:])
                    se64 = spool.tile([128, 1], FP32, tag="se64")
                    nc.vector.tensor_scalar(se64[:], se[:], 1.0 / 64.0, None,
                                            op0=MUL)
                    rd64 = spool.tile([128, 1], FP32, tag="rd64")
                    nc.vector.reciprocal(rd64[:], se64[:])
                    nc.vector.tensor_scalar(msm[:, dg, :], e[:], rd64[:],
                                            None, op0=MUL)
                for t in range(2):
                    for eg in range(G):
                        nc.tensor.matmul(
                            at_ps[eg][:],
                            wv[:, 2 * t:2 * t + 2, eg * 128:(eg + 1) * 128],
                            msm[:, 2 * t:2 * t + 2, :],
                            start=(t == 0), stop=(t == 1), perf_mode=DR)
                # at8 on DVE so the ACT Copy table reload is off the path
                for eg in range(G):
                    nc.vector.tensor_scalar(at8[:, eg, :], at_ps[eg][:],
                                            1.0 / 16.0, None, op0=MUL)

                # ---- row-L1 sums + final per-row scale -------------------
                s_list = []
                for cg in range(G):
                    s_ps = psB.tile([128, 1], FP32, tag="g_ps", bufs=G,
                                    name=f"s_ps{cg}")
                    for dg in range(G):
                        nc.tensor.matmul(
                            s_ps[:],
                            msm[:, dg, cg * 128:(cg + 1) * 128],
                            ones8[:], start=(dg == 0), stop=(dg == G - 1))
                    s_list.append(s_ps)
                for cg in range(G):
                    speps = spool.tile([128, 1], FP32, tag="speps")
                    nc.vector.tensor_scalar(speps[:], s_list[cg][:],
                                            64.0 * EPS, None, op0=ADD)
                    rs = spool.tile([128, 1], FP32, tag="rs")
                    nc.vector.reciprocal(rs[:], speps[:])
                    f = spool.tile([128, 1], FP32, tag="f", bufs=G,
                                   name=f"f{cg}")
                    nc.vector.tensor_tensor(f[:], rs[:], gamma_col[:], op=MUL)
                    fcols.append(f)

            # ---- phase 2: out = A X (fp8 DR); y = x + f*out in one op ----
            with tc.tile_pool(name="ps2", bufs=1, space="PSUM") as ps2:
                for j in range(NJ):
                    ofin = opool.tile([128, G, 512], BF16, tag="ofin", bufs=3,
                                      name=f"ofin{j}")
                    for cg in range(G):
                        o_ps = ps2.tile([128, 512], FP32, tag="o_ps", bufs=6,
                                        name=f"o_ps{j}_{cg}")
                        for t in range(2):
                            nc.tensor.matmul(
                                o_ps[:],
                                at8[:, 2 * t:2 * t + 2,
                                    cg * 128:(cg + 1) * 128],
                                xh8[:, 2 * t:2 * t + 2,
                                    j * 512:(j + 1) * 512],
                                start=(t == 0), stop=(t == 1), perf_mode=DR)
                        if (j * G + cg) % 3 != 0:
                            # ACT scales psum -> bf16; DVE adds in 2x 16-bit
                            osc = opool.tile([128, 512], BF16, tag="osc",
                                             bufs=4, name=f"osc{j}_{cg}")
                            nc.scalar.activation(osc[:], o_ps[:], Copy,
                                                 scale=fcols[cg][:])
                            nc.vector.tensor_tensor(
                                ofin[:, cg, :], osc[:],
                                xhb[:, cg, j * 512:(j + 1) * 512], op=ADD)
                        else:
                            nc.vector.scalar_tensor_tensor(
                                ofin[:, cg, :], o_ps[:], fcols[cg][:],
                                xhb[:, cg, j * 512:(j + 1) * 512],
                                op0=MUL, op1=ADD)
                    # y stores on the Activation HWDGE queue (SP is busy)
                    if j == NJ - 1:
                        for cg in range(G):
                            nc.scalar.dma_start(
                                y_v[:, cg, j * 512:(j + 1) * 512],
                                ofin[:, cg, :])
                    else:
                        nc.scalar.dma_start(y_v[:, :, j * 512:(j + 1) * 512],
                                            ofin[:])

    nc.compile()
    return nc


def _get_nc():
    if "nc" not in _CACHE:
        _CACHE["nc"] = _build_nc()
    return _CACHE["nc"]


def _f8(a):
    return np.clip(a, -F8MAX, F8MAX).astype(ml_dtypes.float8_e4m3)


def _make_in_maps(x, Wq, Wk, Wv, gamma):
    xb = np.ascontiguousarray(x.reshape(B, C, N).astype(np.float32))
    xh8 = _f8(xb)
    xhb = xb.astype(ml_dtypes.bfloat16)
    xt8 = np.ascontiguousarray(np.clip(xb.transpose(0, 2, 1), -F8MAX, F8MAX)
                               .astype(ml_dtypes.float8_e4m3))
    wq8 = _f8(np.ascontiguousarray(16.0 * Wq.T))
    wk8 = _f8(np.ascontiguousarray(16.0 * Wk.T))
    wv8 = _f8(16.0 * np.asarray(Wv, np.float32))
    gcol = np.full((128, 1), float(np.asarray(gamma).reshape(-1)[0]),
                   np.float32)
    ident = np.eye(128, dtype=ml_dtypes.float8_e4m3)
    maps = []
    for i in range(B):
        maps.append({
            "xt": xt8[i], "xh8": xh8[i], "xhb": xhb[i],
            "wq8": wq8, "wk8": wk8, "wv8": wv8,
            "gamma_col": gcol, "ident8": ident,
        })
    return maps


def kernel(x, Wq, Wk, Wv, gamma, _trace=False, _trace_kwargs=None):
    nc = _get_nc()
    in_maps = _make_in_maps(np.asarray(x), np.asarray(Wq), np.asarray(Wk),
                            np.asarray(Wv), np.asarray(gamma))
    kwargs = {}
    if _trace:
        kwargs = dict(trace=True, **(_trace_kwargs or {}))
    res = bass_utils.run_bass_kernel_spmd(nc, in_maps,
                                          core_ids=list(range(B)), **kwargs)
    y = np.stack([np.asarray(res.results[i]["y"], np.float32)
                  .reshape(C, HH, WW) for i in range(B)])
    if _trace:
        kernel._last_result = res
    return y


# revision 32
# speedup vs baseline: 1.1376x; 1.1376x over previous
"""Trainium2 Bass kernel for the contextual channel-attention transformer block.

Contract: kernel(**inputs) takes the FULL unsharded inputs
(x: (8,512,64,64) f32, Wq/Wk/Wv: (512,512) f32, gamma: (1,) f32) and
returns the FULL (8,512,64,64) f32 output.  Internally the batch is
data-parallel across 8 NeuronCores (one batch element per core).

Per-core algorithm (fp8-e4m3 DoubleRow matmuls, fp32 PSUM accumulation):
  Gx   = X @ X.T     upper-triangular blocks fp8 DR + 6 PE transposes
  M3q  = Gx Wq^T, M3k = Gx Wk^T   (fp8 DR)   psum = M3/2
  |Q|^2, |K|^2 = diag(W M3) via ones-matmul colsums
  m3q' = (M3q/128) * rq[c]  (rq = 1/(sqrt8 |Q|) row-broadcast)
  G^T  = (16Wk) m3q' (fp8 DR) = G^T rq/8 ; softmax w/ rk folded into
  ACT scale/bias ; msm8 = 64*softmax ; A^T = (16Wv)^T msm8 (fp8 DR)
  out  = at8 @ X8 (fp8 DR) = 64*out ; y = x_bf16 + f*out in one DVE op
"""

import os
import sys

for _p in ("/opt/trn_rl_repo", "/root/.axon_site/_ro/trn_rl_repo"):
    if os.path.isdir(_p) and _p not in sys.path:
        sys.path.insert(0, _p)

import ml_dtypes
import numpy as np

import concourse.bass as bass
import concourse.tile as tile
from concourse import bacc, bass_utils, mybir

B, C, HH, WW = 8, 512, 64, 64
N = HH * WW          # 4096 spatial positions
G = C // 128         # 4 channel groups of 128
NP = N // 256        # 16 spatial pair-chunks (2x128 for DoubleRow)
NJ = N // 512        # 8 output chunks (512 spatial each)
EPS = 1e-6
FP32 = mybir.dt.float32
BF16 = mybir.dt.bfloat16
F8 = mybir.dt.float8e4
F8MAX = 240.0        # TRN FP8_EXP4 max normal
DR = mybir.MatmulPerfMode.DoubleRow
XT_CHUNKS = [(0, 2), (2, 2), (4, 4), (8, 8)]  # (pair_start, npairs)

_CACHE = {}


def _build_nc():
    nc = bacc.Bacc("TRN2", target_bir_lowering=False)

    xt_d = nc.dram_tensor("xt", [N, C], F8, kind="ExternalInput")     # X^T fp8
    xh8_d = nc.dram_tensor("xh8", [C, N], F8, kind="ExternalInput")   # X fp8
    xhb_d = nc.dram_tensor("xhb", [C, N], BF16, kind="ExternalInput")
    wq_d = nc.dram_tensor("wq8", [C, C], F8, kind="ExternalInput")    # 16Wq^T
    wk_d = nc.dram_tensor("wk8", [C, C], F8, kind="ExternalInput")    # 16Wk^T
    wv_d = nc.dram_tensor("wv8", [C, C], F8, kind="ExternalInput")    # 16Wv
    gcol_d = nc.dram_tensor("gamma_col", [128, 1], FP32, kind="ExternalInput")
    id_d = nc.dram_tensor("ident8", [128, 128], F8, kind="ExternalInput")
    y_d = nc.dram_tensor("y", [C, N], BF16, kind="ExternalOutput")

    xt_v = xt_d.ap().rearrange("(i p) c -> p i c", p=128)     # [128, 32, C]
    xh8_v = xh8_d.ap().rearrange("(g p) n -> p g n", p=128)   # [128, G, N]
    xhb_v = xhb_d.ap().rearrange("(g p) n -> p g n", p=128)
    wq_v = wq_d.ap().rearrange("(g p) o -> p g o", p=128)     # [128, G, C]
    wk_v = wk_d.ap().rearrange("(g p) o -> p g o", p=128)
    wv_v = wv_d.ap().rearrange("(g p) o -> p g o", p=128)
    y_v = y_d.ap().rearrange("(g p) n -> p g n", p=128)

    MUL = mybir.AluOpType.mult
    ADD = mybir.AluOpType.add
    MIN = mybir.AluOpType.min
    AX = mybir.AxisListType.X
    Exp = mybir.ActivationFunctionType.Exp
    Ln = mybir.ActivationFunctionType.Ln
    Copy = mybir.ActivationFunctionType.Copy

    with tile.TileContext(nc) as tc:
        with (
            tc.tile_pool(name="consts", bufs=1) as cpool,
            tc.tile_pool(name="weights", bufs=1) as wpool,
            tc.tile_pool(name="xt", bufs=1) as xtpool,
            tc.tile_pool(name="xbig", bufs=1) as xbigpool,
            tc.tile_pool(name="gram", bufs=1) as gpool,
            tc.tile_pool(name="small", bufs=2) as spool,
            tc.tile_pool(name="mid", bufs=3) as mpool,
            tc.tile_pool(name="msm", bufs=1) as msmpool,
            tc.tile_pool(name="outs", bufs=4) as opool,
        ):
            # ---- input DMAs on SP queue (ident first for PE warmup) ------
            ident = cpool.tile([128, 128], F8, tag="ident")
            nc.sync.dma_start(ident[:], id_d.ap())
            xtp = []
            for ci, (p0, np_) in enumerate(XT_CHUNKS):
                t = xtpool.tile([128, 2 * np_, C], F8, tag=f"xt{ci}",
                                name=f"xt{ci}")
                nc.sync.dma_start(t[:], xt_v[:, 2 * p0:2 * (p0 + np_), :])
                xtp.append(t)

            def xt_pair(i):
                """AP [128, 2, C] for spatial pair i."""
                for (p0, np_), t in zip(XT_CHUNKS, xtp):
                    if p0 <= i < p0 + np_:
                        return t[:, 2 * (i - p0):2 * (i - p0) + 2, :]
                raise AssertionError(i)

            wq = wpool.tile([128, G, C], F8, tag="wq")
            wk = wpool.tile([128, G, C], F8, tag="wk")
            wv = wpool.tile([128, G, C], F8, tag="wv")
            nc.sync.dma_start(wq[:], wq_v)
            nc.sync.dma_start(wk[:], wk_v)
            nc.sync.dma_start(wv[:], wv_v)

            xh8 = xbigpool.tile([128, G, N], F8, tag="xh8")
            nc.sync.dma_start(xh8[:], xh8_v)
            gamma_col = cpool.tile([128, 1], FP32, tag="gamma_col")
            nc.sync.dma_start(gamma_col[:], gcol_d.ap())
            xhb = xbigpool.tile([128, G, N], BF16, tag="xhb")
            nc.sync.dma_start(xhb[:], xhb_v)

            # ---- on-device constants + PE warmup -------------------------
            ones_col = cpool.tile([128, 1], BF16, tag="ones_col")
            nc.vector.memset(ones_col[:], 1.0)
            ones8 = cpool.tile([128, 1], F8, tag="ones8")
            nc.vector.memset(ones8[:], 1.0)
            ones_row = cpool.tile([1, C], BF16, tag="ones_row")
            nc.vector.memset(ones_row[:], 1.0)
            # per-partition Ln input scale: row 0 (sqq) 1.0, row 32 (sqk) 1/4096
            scale_col = cpool.tile([33, 1], FP32, tag="scale_col")
            nc.vector.memset(scale_col[:], 1.0)
            nc.vector.memset(scale_col[32:33, :], 1.0 / 4096.0)
            # -0.25 column: transpose rhs scale so z_ps = -zrow/4 directly
            negq = cpool.tile([128, 1], BF16, tag="negq")
            nc.vector.memset(negq[:], -0.25)

            # ---- Gram: Gx = X X^T upper blocks, fp8 DR, + transposes -----
            # Pair-major head (tracks xt DMA arrival), bank-major tail with
            # pipelined PSUM->SBUF copies and PE transposes for lower blocks.
            gx8 = gpool.tile([128, G, C], F8, tag="gx8")
            NHEAD = 8
            with tc.tile_pool(name="psG", bufs=1, space="PSUM") as psG:
                scr = psG.tile([128, 128], FP32, tag="scr", name="warm")
                for i in range(24):
                    nc.tensor.matmul(scr[:], ident[:], ident[:],
                                     start=(i == 0), stop=(i == 23))
                gx_ps = [psG.tile([128, C], FP32, tag=f"gx{cg}", bufs=1,
                                  name=f"gx{cg}") for cg in range(G)]

                def gram_mm(cg, i):
                    nc.tensor.matmul(
                        gx_ps[cg][:, cg * 128:],
                        xt_pair(i)[:, :, cg * 128:(cg + 1) * 128],
                        xt_pair(i)[:, :, cg * 128:],
                        start=(i == 0), stop=(i == NP - 1), perf_mode=DR)

                def tp_block(dg, cg, eng):
                    tp_ps = psG.tile([128, 256], F8, tag="tp", bufs=3,
                                     name=f"tp{dg}_{cg}")
                    nc.tensor.transpose(
                        tp_ps[:, 0:256:2],
                        gx8[:, cg, dg * 128:(dg + 1) * 128], ident[:])
                    if eng == "act":
                        nc.scalar.activation(
                            gx8[:, dg, cg * 128:(cg + 1) * 128],
                            tp_ps[:, 0:256:2], Copy)
                    else:
                        nc.vector.tensor_copy(
                            gx8[:, dg, cg * 128:(cg + 1) * 128],
                            tp_ps[:, 0:256:2])

                for i in range(NHEAD):
                    for cg in range(G):
                        gram_mm(cg, i)
                for cg in range(G):
                    for i in range(NHEAD, NP):
                        gram_mm(cg, i)
                    nc.scalar.activation(gx8[:, cg, cg * 128:],
                                         gx_ps[cg][:, cg * 128:], Copy,
                                         scale=1.0 / 32.0)
                    # transposes whose source row is now complete
                    for dg in range(cg + 1, G):
                        if dg == cg + 1:
                            for c2 in range(dg):
                                tp_block(dg, c2, "act" if c2 % 2 else "dve")

            m3q16 = gpool.tile([128, G, C], BF16, tag="m3q16")
            m3q8 = gpool.tile([128, G, C], F8, tag="m3q8")
            tqs, tks = [], []

            with tc.tile_pool(name="psN", bufs=1, space="PSUM") as psN:
                # packed norms: row 0 = sqq = 8|Q|^2, row 32 = sqk = 8|K|^2
                nsum = psN.tile([33, C], FP32, tag="nsum", name="nsum")
                sqq = nsum[0:1, :]
                sqk = nsum[32:33, :]

                # ---- M3q = Gx Wq^T (psum = M3q/2) ------------------------
                with tc.tile_pool(name="psQ", bufs=1, space="PSUM") as psQ:
                    for cg in range(G):
                        q_ps = psQ.tile([128, C], FP32, tag="q", bufs=G,
                                        name=f"q{cg}")
                        for t in range(2):
                            nc.tensor.matmul(
                                q_ps[:],
                                gx8[:, 2 * t:2 * t + 2,
                                    cg * 128:(cg + 1) * 128],
                                wq[:, 2 * t:2 * t + 2, :],
                                start=(t == 0), stop=(t == 1), perf_mode=DR)
                        tq = mpool.tile([128, C], BF16, tag="tq", bufs=G,
                                        name=f"tq{cg}")
                        nc.vector.tensor_tensor(tq[:], wq[:, cg, :], q_ps[:],
                                                op=MUL)
                        tqs.append(tq)
                        nc.scalar.activation(m3q16[:, cg, :], q_ps[:], Copy,
                                             scale=1.0 / 64.0)
                    for cg in range(G):
                        nc.tensor.matmul(sqq, ones_col[:], tqs[cg][:],
                                         start=(cg == 0), stop=(cg == G - 1))

                # ---- M3k = Gx Wk^T; tk for |K|^2 -------------------------
                with tc.tile_pool(name="psK", bufs=1, space="PSUM") as psK:
                    for cg in range(G):
                        k_ps = psK.tile([128, C], FP32, tag="k", bufs=3,
                                        name=f"k{cg}")
                        for t in range(2):
                            nc.tensor.matmul(
                                k_ps[:],
                                gx8[:, 2 * t:2 * t + 2,
                                    cg * 128:(cg + 1) * 128],
                                wk[:, 2 * t:2 * t + 2, :],
                                start=(t == 0), stop=(t == 1), perf_mode=DR)
                        tk = mpool.tile([128, C], BF16, tag="tk", bufs=G,
                                        name=f"tk{cg}")
                        nc.vector.tensor_tensor(tk[:], wk[:, cg, :], k_ps[:],
                                                op=MUL)
                        tks.append(tk)
                    for cg in range(G):
                        nc.tensor.matmul(sqk, ones_col[:], tks[cg][:],
                                         start=(cg == 0), stop=(cg == G - 1))

                # single Ln + single Exp over the packed [33, C] tile
                lnp = spool.tile([33, C], FP32, tag="lnp")
                nc.scalar.activation(lnp[:], nsum[:], Ln, scale=scale_col[:])
                ep = spool.tile([33, C], BF16, tag="ep")
                nc.scalar.activation(ep[:], lnp[:], Exp, scale=-0.5)
                rq_bf = ep[0:1, :]       # 1/(sqrt8 |Q|)
                zrow = ep[32:33, :]      # 64*rk = 22.6/|K|

                zs = []
                with tc.tile_pool(name="psS", bufs=1, space="PSUM") as psS:
                    bq_ps = psS.tile([128, C], FP32, tag="bq_ps", name="bq_ps")
                    nc.tensor.matmul(bq_ps[:], ones_row[:, 0:128], rq_bf,
                                     start=True, stop=True)
                    # zneg columns (-16*rk per d) via scaled PE transposes
                    for dg in range(G):
                        z_ps = psS.tile([128, 1], BF16, tag="ztp", bufs=2,
                                        name=f"ztp{dg}")
                        nc.tensor.transpose(
                            z_ps[:], zrow[:, dg * 128:(dg + 1) * 128],
                            negq[32:33, 0:1])
                        z = spool.tile([128, 1], FP32, tag="z", bufs=G,
                                       name=f"z{dg}")
                        nc.vector.tensor_copy(z[:], z_ps[:])
                        zs.append(z)

                    # m3q8 = (M3q/128) * rq[c]  (fp8, col-scaled; bq from PSUM)
                    for cg in range(G):
                        nc.vector.tensor_tensor(m3q8[:, cg, :],
                                                m3q16[:, cg, :],
                                                bq_ps[:], op=MUL)

            # ---- per-dg: G^T -> softmax -> msm8; A^T over dg pairs -------
            msm = msmpool.tile([128, G, C], F8, tag="msm")
            at8 = msmpool.tile([128, G, C], F8, tag="at8")
            fcols = []
            with tc.tile_pool(name="psB", bufs=1, space="PSUM") as psB:
                at_ps = [psB.tile([128, C], FP32, tag="at", bufs=G,
                                  name=f"at{eg}") for eg in range(G)]
                for dg in range(G):
                    g_ps = psB.tile([128, C], FP32, tag="g_ps", bufs=G,
                                    name=f"g_ps{dg}")
                    for t in range(2):
                        nc.tensor.matmul(
                            g_ps[:],
                            wk[:, 2 * t:2 * t + 2, dg * 128:(dg + 1) * 128],
                            m3q8[:, 2 * t:2 * t + 2, :],
                            start=(t == 0), stop=(t == 1), perf_mode=DR)
                    # zneg = -z/4; den4 = (1+eps-mn0*z)/4 = mn0*zneg + (1+eps)/4
                    mn0 = spool.tile([128, 1], FP32, tag="mn0")
                    nc.vector.tensor_reduce(mn0[:], g_ps[:], axis=AX, op=MIN)
                    den4 = spool.tile([128, 1], FP32, tag="den4")
                    nc.vector.tensor_scalar(den4[:], mn0[:], zs[dg][:],
                                            0.25 * (1.0 + EPS),
                                            op0=MUL, op1=ADD)
                    r4 = spool.tile([128, 1], FP32, tag="r4")
                    nc.vector.reciprocal(r4[:], den4[:])
                    sv = spool.tile([128, 1], FP32, tag="sv")
                    nc.vector.tensor_scalar(sv[:], r4[:], zs[dg][:], -4.0,
                                            op0=MUL, op1=MUL)
                    bv = spool.tile([128, 1], FP32, tag="bv")
                    nc.vector.tensor_scalar(bv[:], r4[:], -1.0, 1.0,
                                            op0=MUL, op1=ADD)
                    e = mpool.tile([128, C], BF16, tag="e")
                    se = spool.tile([128, 1], FP32, tag="se")
                    nc.scalar.activation(e[:], g_ps[:], Exp,
                                         bias=bv[:], scale=sv[:],
                                         accum_out=se[:])
                    rd = spool.tile([128, 1], FP32, tag="rd")
                    nc.vector.reciprocal(rd[:], se[:])
                    nc.vector.tensor_scalar(msm[:, dg, :], e[:], rd[:],
                                            64.0, op0=MUL, op1=MUL)
                for t in range(2):
                    for eg in range(G):
                        nc.tensor.matmul(
                            at_ps[eg][:],
                            wv[:, 2 * t:2 * t + 2, eg * 128:(eg + 1) * 128],
                            msm[:, 2 * t:2 * t + 2, :],
                            start=(t == 0), stop=(t == 1), perf_mode=DR)
                # at8 on ACT (idle post-softmax; Copy table reload hidden
                # behind the DVE fcols chain)
                for eg in range(G):
                    nc.scalar.activation(at8[:, eg, :], at_ps[eg][:], Copy,
                                         scale=1.0 / 16.0)

                # ---- row-L1 sums + final per-row scale -------------------
                s_list = []
                for cg in range(G):
                    s_ps = psB.tile([128, 1], FP32, tag="g_ps", bufs=G,
                                    name=f"s_ps{cg}")
                    for dg in range(G):
                        nc.tensor.matmul(
                            s_ps[:],
                            msm[:, dg, cg * 128:(cg + 1) * 128],
                            ones8[:], start=(dg == 0), stop=(dg == G - 1))
                    s_list.append(s_ps)
                for cg in range(G):
                    speps = spool.tile([128, 1], FP32, tag="speps")
                    nc.vector.tensor_scalar(speps[:], s_list[cg][:],
                                            64.0 * EPS, None, op0=ADD)
                    rs = spool.tile([128, 1], FP32, tag="rs")
                    nc.vector.reciprocal(rs[:], speps[:])
                    f = spool.tile([128, 1], FP32, tag="f", bufs=G,
                                   name=f"f{cg}")
                    nc.vector.tensor_tensor(f[:], rs[:], gamma_col[:], op=MUL)
                    fcols.append(f)

            # ---- phase 2: out = A X (fp8 DR); y = x + f*out in one op ----
            with tc.tile_pool(name="ps2", bufs=1, space="PSUM") as ps2:
                for j in range(NJ):
                    ofin = opool.tile([128, G, 512], BF16, tag="ofin", bufs=3,
                                      name=f"ofin{j}")
                    for cg in range(G):
                        o_ps = ps2.tile([128, 512], FP32, tag="o_ps", bufs=6,
                                        name=f"o_ps{j}_{cg}")
                        for t in range(2):
                            nc.tensor.matmul(
                                o_ps[:],
                                at8[:, 2 * t:2 * t + 2,
                                    cg * 128:(cg + 1) * 128],
                                xh8[:, 2 * t:2 * t + 2,
                                    j * 512:(j + 1) * 512],
                                start=(t == 0), stop=(t == 1), perf_mode=DR)
                        if (j * G + cg) % 3 != 0:
                            # ACT scales psum -> bf16; DVE adds in 2x 16-bit
                            osc = opool.tile([128, 512], BF16, tag="osc",
                                             bufs=4, name=f"osc{j}_{cg}")
                            nc.scalar.activation(osc[:], o_ps[:], Copy,
                                                 scale=fcols[cg][:])
                            nc.vector.tensor_tensor(
                                ofin[:, cg, :], osc[:],
                                xhb[:, cg, j * 512:(j + 1) * 512], op=ADD)
                        else:
                            nc.vector.scalar_tensor_tensor(
                                ofin[:, cg, :], o_ps[:], fcols[cg][:],
                                xhb[:, cg, j * 512:(j + 1) * 512],
                                op0=MUL, op1=ADD)
                    # y stores on the Activation HWDGE queue (SP is busy)
                    if j == NJ - 1:
                        for cg in range(G):
                            nc.scalar.dma_start(
                                y_v[:, cg, j * 512:(j + 1) * 512],
                                ofin[:, cg, :])
                    else:
                        nc.scalar.dma_start(y_v[:, :, j * 512:(j + 1) * 512],
                                            ofin[:])

    nc.compile()
    return nc


def _get_nc():
    if "nc" not in _CACHE:
        _CACHE["nc"] = _build_nc()
    return _CACHE["nc"]


def _f8(a):
    return np.clip(a, -F8MAX, F8MAX).astype(ml_dtypes.float8_e4m3)


def _make_in_maps(x, Wq, Wk, Wv, gamma):
    xb = np.ascontiguousarray(x.reshape(B, C, N).astype(np.float32))
    xh8 = _f8(xb)
    xhb = xb.astype(ml_dtypes.bfloat16)
    xt8 = np.ascontiguousarray(np.clip(xb.transpose(0, 2, 1), -F8MAX, F8MAX)
                               .astype(ml_dtypes.float8_e4m3))
    wq8 = _f8(np.ascontiguousarray(16.0 * Wq.T))
    wk8 = _f8(np.ascontiguousarray(16.0 * Wk.T))
    wv8 = _f8(16.0 * np.asarray(Wv, np.float32))
    gcol = np.full((128, 1), float(np.asarray(gamma).reshape(-1)[0]),
                   np.float32)
    ident = np.eye(128, dtype=ml_dtypes.float8_e4m3)
    maps = []
    for i in range(B):
        maps.append({
            "xt": xt8[i], "xh8": xh8[i], "xhb": xhb[i],
            "wq8": wq8, "wk8": wk8, "wv8": wv8,
            "gamma_col": gcol, "ident8": ident,
        })
    return maps


def kernel(x, Wq, Wk, Wv, gamma, _trace=False, _trace_kwargs=None):
    nc = _get_nc()
    in_maps = _make_in_maps(np.asarray(x), np.asarray(Wq), np.asarray(Wk),
                            np.asarray(Wv), np.asarray(gamma))
    kwargs = {}
    if _trace:
        kwargs = dict(trace=True, **(_trace_kwargs or {}))
    res = bass_utils.run_bass_kernel_spmd(nc, in_maps,
                                          core_ids=list(range(B)), **kwargs)
    y = np.stack([np.asarray(res.results[i]["y"], np.float32)
                  .reshape(C, HH, WW) for i in range(B)])
    if _trace:
        kernel._last_result = res
    return y
